# revision 9
# baseline (speedup 1.0000x reference)
import sys, os

sys.path.insert(0, "/opt/trn_rl_repo")
import numpy as np

import concourse.bass as bass
import concourse.mybir as mybir
import concourse.bacc as bacc
import concourse.tile as tile

NCORES = 8
B = 8  # images per core
F16, F32 = mybir.dt.float16, mybir.dt.float32
AF, ALU = mybir.ActivationFunctionType, mybir.AluOpType
BETA = 0.5
BN_EPS = 1e-5

TRIPLES = [(0, 3), (3, 3), (6, 2)]
# conv2 5x5 shift pairing: ((khA,kwA),(khB,kwB), which shifted-copy tensor)
# 'b': partitions 64..127 hold input shifted by +1 flat element
# 'c': partitions 64..127 hold input shifted by +27 flat elements (= +1 row -4 cols)
PAIRS2 = [
    ((0, 0), (0, 1), 'b'), ((0, 2), (0, 3), 'b'), ((0, 4), (1, 0), 'c'),
    ((1, 1), (1, 2), 'b'), ((1, 3), (1, 4), 'b'),
    ((2, 0), (2, 1), 'b'), ((2, 2), (2, 3), 'b'), ((2, 4), (3, 0), 'c'),
    ((3, 1), (3, 2), 'b'), ((3, 3), (3, 4), 'b'),
    ((4, 0), (4, 1), 'b'), ((4, 2), (4, 3), 'b'),
]  # + single (4,4) at tile index 12
CH1 = [(0, 512), (512, 512), (1024, 512), (1536, 512), (2048, 512), (2560, 465)]
ROWS2 = [(0, 14), (14, 13)]
NSTAT = 224
NOUT = NSTAT + 16

SIZES = np.array([64 * 3 * 224 * 224, 64 * 64 * 27 * 27, 64 * 192 * 13 * 13,
                  64 * 384 * 13 * 13, 64 * 256 * 13 * 13, 64 * 256 * 36,
                  64 * 4096, 64 * 4096], dtype=np.int32)

_g = {}


def _win(t_ap, pstart, pcount, off, dims):
    """Custom AP: partition range + arbitrary (step,count) free dims."""
    pstep = t_ap.ap[0][0]
    return bass.AP(tensor=t_ap.tensor, offset=t_ap.offset + pstart * pstep + off,
                   ap=[[pstep, pcount]] + [list(d) for d in dims])


def _build():
    nc = bacc.Bacc("TRN2", target_bir_lowering=False, debug=False,
                   num_devices=NCORES)
    man = []

    def col(kind, stage, extra=None):
        man.append((kind, stage, extra))
        return len(man) - 1

    def P(n, s, d):
        return nc.dram_tensor(n, s, d, kind="ExternalInput").ap()

    xflat = P("xflat", [128, 9408], F32)
    r1 = P("r1", [B, 3, 121, 3025], F16)
    w1p = P("w1p", [121, 3, 64], F16)
    w2l = P("w2l", [128, 13, 192], F16)
    w3a = P("w3a", [128, 9, 384], F16)
    w3b = P("w3b", [128, 6, 384], F16)
    w4p = P("w4p", [128, 27, 256], F16)
    w5p = P("w5p", [128, 18, 256], F16)
    bconv = P("bconv", [128, 10], F32)
    w1fc = P("w1fc", [72, 128, 512], F16)
    w2fc = P("w2fc", [32, 128, 512], F16)
    w3fc = P("w3fc", [128, 4, 10], F16)
    b1r = P("b1r", [1, 512], F16)
    b2r = P("b2r", [1, 512], F16)
    idm = P("idm", [64, 64], F16)
    out = nc.dram_tensor("out", [128, NOUT], F32, kind="ExternalOutput").ap()

    with tile.TileContext(nc) as tc:
        with (tc.tile_pool(name="wsb", bufs=1) as wsb,
              tc.tile_pool(name="rp", bufs=6) as rp,
              tc.tile_pool(name="s1", bufs=2) as s1p,
              tc.tile_pool(name="p2", bufs=2) as p2p,
              tc.tile_pool(name="s2", bufs=2) as s2p,
              tc.tile_pool(name="p3", bufs=2) as p3p,
              tc.tile_pool(name="p45", bufs=2) as p45p,
              tc.tile_pool(name="scr", bufs=2) as scrp,
              tc.tile_pool(name="fc", bufs=1) as fcp,
              tc.tile_pool(name="wfc", bufs=6) as wfcp,
              tc.tile_pool(name="ps", bufs=4, space="PSUM") as ps,
              tc.tile_pool(name="psf", bufs=1, space="PSUM") as psf,
              tc.tile_pool(name="pst", bufs=1, space="PSUM") as pst,
              tc.tile_pool(name="dram", bufs=1, space="DRAM") as dram):

            # resident weights / consts
            w1sb = wsb.tile([121, 3, 64], F16); nc.sync.dma_start(w1sb[:], w1p)
            w2sb = wsb.tile([128, 13, 192], F16); nc.sync.dma_start(w2sb[:], w2l)
            w3asb = wsb.tile([128, 9, 384], F16); nc.sync.dma_start(w3asb[:], w3a)
            w3bsb = wsb.tile([128, 6, 384], F16); nc.sync.dma_start(w3bsb[:], w3b)
            w4sb = wsb.tile([128, 27, 256], F16); nc.sync.dma_start(w4sb[:], w4p)
            w5sb = wsb.tile([128, 18, 256], F16); nc.sync.dma_start(w5sb[:], w5p)
            bcsb = wsb.tile([128, 10], F32); nc.sync.dma_start(bcsb[:], bconv)
            idsb = wsb.tile([64, 64], F16); nc.sync.dma_start(idsb[:], idm)
            b1sb = wsb.tile([1, 512], F16); nc.sync.dma_start(b1sb[:], b1r)
            b2sb = wsb.tile([1, 512], F16); nc.sync.dma_start(b2sb[:], b2r)
            w3fsb = wsb.tile([128, 4, 10], F16); nc.sync.dma_start(w3fsb[:], w3fc)
            ones1 = wsb.tile([1, 64], F16); nc.vector.memset(ones1[:], 1.0)
            stat = wsb.tile([128, NSTAT], F32); nc.vector.memset(stat[:], 0.0)
            pool5sb = []
            for m in range(2):
                t = wsb.tile([128, B, 36], F16, name=f"pool5_{m}")
                pool5sb.append(t)

            def zstat(ap_in, shape, stage, tag, nm):
                s = scrp.tile(shape, F16, name=f"z_{nm}", tag=tag)
                c = col('z', stage)
                nc.vector.tensor_scalar(s[:], ap_in, 0.0, None, ALU.is_equal,
                                        op1=ALU.add,
                                        accum_out=stat[0:shape[0], c:c + 1])

            def tstat(ap_in, shape, kind, stage, extra, tag, nm):
                s = scrp.tile(shape, F16, name=f"t_{nm}", tag=tag)
                c = col(kind, stage, extra)
                nc.scalar.activation(s[:], ap_in, AF.Tanh, scale=BETA,
                                     accum_out=stat[0:shape[0], c:c + 1])

            # ---- input zero count
            for i in range(4):
                xc = rp.tile([128, 2352], F32, name=f"xc{i}", tag="xc", bufs=2)
                nc.sync.dma_start(xc[:], xflat[:, i * 2352:(i + 1) * 2352])
                zstat(xc[:], [128, 2352], 0, "xscr", f"x{i}")

            # persistent across-loop tiles
            p3t = {}
            p4t = {}
            p5t = {}
            s5t = {}

            for img in range(B):
                pair, lo = img // 2, (img % 2) * 64
                tri = 0 if img < 3 else (1 if img < 6 else 2)
                t0, nimg = TRIPLES[tri]
                iml = img - t0

                # ============ conv1 + pool1 (per pair) ============
                if img % 2 == 0:
                    a, b_ = 2 * pair, 2 * pair + 1
                    rts = {}
                    for im, plo in ((a, 0), (b_, 64)):
                        for c in range(3):
                            rt = rp.tile([121, 3025], F16,
                                         name=f"r_{im}_{c}", tag="rt")
                            nc.sync.dma_start(rt[:], r1[im, c])
                            rts[(plo, c)] = rt
                    s1pair = s1p.tile([128, 3025], F16,
                                      name=f"s1pair{pair}", tag="s1pair")
                    for (nofs, nch) in CH1:
                        pt = ps.tile([128, 512], F32,
                                     name=f"ps1_{pair}_{nofs}", tag="psum")
                        for plo in (0, 64):
                            for c in range(3):
                                nc.tensor.matmul(
                                    pt[plo:plo + 64, 0:nch],
                                    w1sb[:, c, :],
                                    rts[(plo, c)][:, nofs:nofs + nch],
                                    start=(c == 0), stop=(c == 2))
                        nc.scalar.activation(s1pair[:, nofs:nofs + nch],
                                             pt[:, 0:nch], AF.Relu,
                                             bias=bcsb[:, 0:1])
                    v = s1pair[:].rearrange("q (y x) -> q y x", y=55)
                    t1 = scrp.tile([128, 55, 27], F16,
                                   name=f"t1_{pair}", tag="ptmp1")
                    nc.vector.tensor_max(t1[:], v[:, :, 0:53:2], v[:, :, 1:54:2])
                    nc.vector.tensor_max(t1[:], t1[:], v[:, :, 2:55:2])
                    s1pool = s1p.tile([128, 27, 27], F16,
                                      name=f"s1pool{pair}", tag="s1pool")
                    nc.vector.tensor_max(s1pool[:], t1[:, 0:53:2, :],
                                         t1[:, 1:54:2, :])
                    nc.vector.tensor_max(s1pool[:], s1pool[:], t1[:, 2:55:2, :])
                    s1f = s1pool[:].rearrange("q y x -> q (y x)")
                    zstat(s1f, [128, 729], 1, "scr729", f"s1_{pair}")
                    tstat(s1f, [128, 729], 'tp', 1, (a, b_), "scr729",
                          f"s1_{pair}")
                    _g.setdefault('s1pool_tiles', {})[pair] = s1pool

                s1pool = _g['s1pool_tiles'][pair]

                # ============ P2 build (per img) ============
                p2b = p2p.tile([128, 31, 31], F16, name=f"p2b{img}", tag="p2b")
                p2c = p2p.tile([128, 31, 31], F16, name=f"p2c{img}", tag="p2c")
                nc.vector.memset(p2b[:], 0.0)
                nc.vector.memset(p2c[:], 0.0)
                src = s1pool[lo:lo + 64, :, :]
                nc.sync.dma_start(p2b[0:64, 2:29, 2:29], src)
                nc.sync.dma_start(p2c[0:64, 2:29, 2:29], src)
                p2bf = p2b[:].rearrange("q y x -> q (y x)")
                p2cf = p2c[:].rearrange("q y x -> q (y x)")
                nc.sync.dma_start(p2bf[64:128, 0:960], p2bf[0:64, 1:961])
                nc.sync.dma_start(p2cf[64:128, 0:934], p2cf[0:64, 27:961])

                # ============ conv2 + pool2 ============
                s2a = s2p.tile([128, 27, 27], F16, name=f"s2a{img}", tag="s2a")
                s2b = s2p.tile([64, 27, 27], F16, name=f"s2b{img}", tag="s2b")
                for (mlo, msz, mtile, bcol) in ((0, 128, s2a, 1),
                                                (128, 64, s2b, 2)):
                    for (r0, nr) in ROWS2:
                        pt = ps.tile([128, 378], F32,
                                     name=f"ps2_{img}_{mlo}_{r0}", tag="psum")
                        for j, (sA, sB, kind) in enumerate(PAIRS2):
                            khA, kwA = sA
                            off = khA * 31 + kwA + r0 * 31
                            sf = p2bf if kind == 'b' else p2cf
                            nc.tensor.matmul(
                                pt[0:msz, 0:nr * 27],
                                w2sb[:, j, mlo:mlo + msz],
                                _win(sf, 0, 128, off, [(31, nr), (1, 27)]),
                                start=(j == 0), stop=False)
                        off = 4 * 31 + 4 + r0 * 31
                        nc.tensor.matmul(
                            pt[0:msz, 0:nr * 27],
                            w2sb[0:64, 12, mlo:mlo + msz],
                            _win(p2bf, 0, 64, off, [(31, nr), (1, 27)]),
                            start=False, stop=True)
                        nc.scalar.activation(
                            mtile[0:msz, r0:r0 + nr, :],
                            pt[0:msz, 0:nr * 27].rearrange(
                                "q (a b) -> q a b", a=nr),
                            AF.Relu, bias=bcsb[0:msz, bcol:bcol + 1])

                if iml == 0:
                    p3a = p3p.tile([128, nimg, 15, 15], F16,
                                   name=f"p3a_{tri}", tag="p3a")
                    p3b = p3p.tile([128, nimg, 15, 15], F16,
                                   name=f"p3b_{tri}", tag="p3b")
                    nc.vector.memset(p3a[:], 0.0)
                    nc.vector.memset(p3b[:], 0.0)
                    p3t[tri] = (p3a, p3b)
                p3a, p3b = p3t[tri]

                for (mtile, msz, p3x) in ((s2a, 128, p3a), (s2b, 64, p3b)):
                    t2 = scrp.tile([128, 27, 13], F16,
                                   name=f"t2_{img}_{msz}", tag="ptmp2")
                    nc.vector.tensor_max(t2[0:msz], mtile[:, :, 0:25:2],
                                         mtile[:, :, 1:26:2])
                    nc.vector.tensor_max(t2[0:msz], t2[0:msz],
                                         mtile[:, :, 2:27:2])
                    dst = p3x[0:msz, iml, 1:14, 1:14]
                    nc.vector.tensor_max(dst, t2[0:msz, 0:25:2, :],
                                         t2[0:msz, 1:26:2, :])
                    nc.vector.tensor_max(dst, dst, t2[0:msz, 2:27:2, :])
                    iv = p3x[0:msz, iml, 1:14, 1:14]
                    zstat(iv, [msz, 13, 13], 2, "scr169", f"s2_{img}_{msz}")
                    tstat(iv, [msz, 13, 13], 'ts', 2, img, "scr169",
                          f"s2_{img}_{msz}")

                # ============ conv3/4/5 at triple end ============
                if iml == nimg - 1:
                    p3bf = p3b[:].rearrange("q a y x -> q (a y x)")
                    nc.sync.dma_start(p3bf[64:128, 0:nimg * 225 - 1],
                                      p3bf[0:64, 1:nimg * 225])
                    p3af = p3a[:].rearrange("q a y x -> q (a y x)")
                    AP3 = [(225, nimg), (15, 13), (1, 13)]
                    N3 = nimg * 169

                    # conv3
                    p4s = []
                    for m in range(3):
                        p4x = p45p.tile([128, nimg, 15, 15], F16,
                                        name=f"p4_{tri}_{m}", tag=f"p4_{m}")
                        nc.vector.memset(p4x[:], 0.0)
                        p4s.append(p4x)
                    for m in range(3):
                        pt = ps.tile([128, 512], F32,
                                     name=f"ps3_{tri}_{m}", tag="psum")
                        ms = slice(m * 128, (m + 1) * 128)
                        for kh in range(3):
                            for kw in range(3):
                                nc.tensor.matmul(
                                    pt[:, 0:N3], w3asb[:, kh * 3 + kw, ms],
                                    _win(p3af, 0, 128, kh * 15 + kw, AP3),
                                    start=(kh == 0 and kw == 0), stop=False)
                        for kh in range(3):
                            nc.tensor.matmul(
                                pt[:, 0:N3], w3bsb[:, kh * 2, ms],
                                _win(p3bf, 0, 128, kh * 15 + 0, AP3),
                                start=False, stop=False)
                            nc.tensor.matmul(
                                pt[:, 0:N3], w3bsb[0:64, kh * 2 + 1, ms],
                                _win(p3bf, 0, 64, kh * 15 + 2, AP3),
                                start=False, stop=(kh == 2))
                        nc.scalar.activation(
                            p4s[m][:, :, 1:14, 1:14],
                            pt[:, 0:N3].rearrange("q (a y x) -> q a y x",
                                                  a=nimg, y=13),
                            AF.Relu, bias=bcsb[:, 3 + m:4 + m])
                        for i2 in range(nimg):
                            iv = p4s[m][:, i2, 1:14, 1:14]
                            zstat(iv, [128, 13, 13], 3, "scr169",
                                  f"s3_{tri}_{m}_{i2}")
                            tstat(iv, [128, 13, 13], 'ts', 3, t0 + i2,
                                  "scr169", f"s3_{tri}_{m}_{i2}")

                    # conv4
                    p5s = []
                    for m in range(2):
                        p5x = p45p.tile([128, nimg, 15, 15], F16,
                                        name=f"p5_{tri}_{m}", tag=f"p5_{m}")
                        nc.vector.memset(p5x[:], 0.0)
                        p5s.append(p5x)
                    p4fs = [p[:].rearrange("q a y x -> q (a y x)")
                            for p in p4s]
                    for m in range(2):
                        pt = ps.tile([128, 512], F32,
                                     name=f"ps4_{tri}_{m}", tag="psum")
                        ms = slice(m * 128, (m + 1) * 128)
                        n = 0
                        for cb in range(3):
                            for kh in range(3):
                                for kw in range(3):
                                    nc.tensor.matmul(
                                        pt[:, 0:N3],
                                        w4sb[:, cb * 9 + kh * 3 + kw, ms],
                                        _win(p4fs[cb], 0, 128,
                                             kh * 15 + kw, AP3),
                                        start=(n == 0), stop=(n == 26))
                                    n += 1
                        nc.scalar.activation(
                            p5s[m][:, :, 1:14, 1:14],
                            pt[:, 0:N3].rearrange("q (a y x) -> q a y x",
                                                  a=nimg, y=13),
                            AF.Relu, bias=bcsb[:, 6 + m:7 + m])
                        for i2 in range(nimg):
                            iv = p5s[m][:, i2, 1:14, 1:14]
                            zstat(iv, [128, 13, 13], 4, "scr169",
                                  f"s4_{tri}_{m}_{i2}")
                            tstat(iv, [128, 13, 13], 'ts', 4, t0 + i2,
                                  "scr169", f"s4_{tri}_{m}_{i2}")

                    # conv5 + pool5
                    p5fs = [p[:].rearrange("q a y x -> q (a y x)") for p in p5s]
                    for m in range(2):
                        pt = ps.tile([128, 512], F32,
                                     name=f"ps5_{tri}_{m}", tag="psum")
                        ms = slice(m * 128, (m + 1) * 128)
                        n = 0
                        for cb in range(2):
                            for kh in range(3):
                                for kw in range(3):
                                    nc.tensor.matmul(
                                        pt[:, 0:N3],
                                        w5sb[:, cb * 9 + kh * 3 + kw, ms],
                                        _win(p5fs[cb], 0, 128,
                                             kh * 15 + kw, AP3),
                                        start=(n == 0), stop=(n == 17))
                                    n += 1
                        s5 = s2p.tile([128, nimg, 13, 13], F16,
                                      name=f"s5_{tri}_{m}", tag="s5")
                        nc.scalar.activation(
                            s5[:], pt[:, 0:N3].rearrange(
                                "q (a y x) -> q a y x", a=nimg, y=13),
                            AF.Relu, bias=bcsb[:, 8 + m:9 + m])
                        for i2 in range(nimg):
                            vv = s5[:, i2]
                            t5 = scrp.tile([128, 13, 6], F16,
                                           name=f"t5_{tri}_{m}_{i2}",
                                           tag="ptmp5")
                            nc.vector.tensor_max(t5[:], vv[:, :, 0:11:2],
                                                 vv[:, :, 1:12:2])
                            nc.vector.tensor_max(t5[:], t5[:], vv[:, :, 2:13:2])
                            dst = pool5sb[m][:, t0 + i2, :].rearrange(
                                "q (y x) -> q y x", y=6)
                            nc.vector.tensor_max(dst, t5[:, 0:11:2, :],
                                                 t5[:, 1:12:2, :])
                            nc.vector.tensor_max(dst, dst, t5[:, 2:13:2, :])
                            pv = pool5sb[m][:, t0 + i2, :]
                            zstat(pv, [128, 36], 5, "scr36",
                                  f"s5_{tri}_{m}_{i2}")
                            tstat(pv, [128, 36], 'ts', 5, t0 + i2, "scr36",
                                  f"s5_{tri}_{m}_{i2}")

            # ============ FC ============
            ag1in = dram.tile([2, 128, B, 36], F16)
            for m in range(2):
                nc.sync.dma_start(ag1in[m], pool5sb[m][:])
            ag1out = dram.tile([NCORES, 2, 128, B, 36], F16,
                               addr_space="Shared")
            nc.gpsimd.collective_compute(
                "AllGather", ALU.bypass,
                replica_groups=[list(range(NCORES))],
                ins=[ag1in[:].opt()], outs=[ag1out[:].opt()])

            # xfc layout: [c_part, (cb, rank, img, s)] so the post-gather
            # copy is plain contiguous; fc1 lhsT chunks are strided views.
            xfc = fcp.tile([128, 2, NCORES, B, 36], F16)
            ag1b = ag1out[:]
            xb = xfc[:]
            for cb in range(2):
                srcap = bass.AP(
                    tensor=ag1b.tensor,
                    offset=ag1b.offset + cb * 128 * B * 36,
                    ap=[[B * 36, 128], [2 * 128 * B * 36, NCORES],
                        [1, B * 36]])
                dstap = bass.AP(
                    tensor=xb.tensor,
                    offset=xb.offset + cb * NCORES * B * 36,
                    ap=[[xb.ap[0][0], 128], [B * 36, NCORES], [1, B * 36]])
                nc.sync.dma_start(out=dstap, in_=srcap)

            psf1 = psf.tile([64, 512], F32, name="psf1", tag="psumfc")
            for kc in range(72):
                s, cb = kc // 2, kc % 2
                wt = wfcp.tile([128, 512], F16, name=f"w1t{kc}", tag="w1t",
                               bufs=4)
                nc.sync.dma_start(wt[:], w1fc[kc])
                lhs = _win(xb, 0, 128, cb * NCORES * B * 36 + s,
                           [(B * 36, NCORES), (36, B)])
                nc.tensor.matmul(psf1[:], lhs, wt[:],
                                 start=(kc == 0), stop=False)
            nc.tensor.matmul(psf1[:], ones1[:], b1sb[:], start=False, stop=True)
            a2 = fcp.tile([64, 512], F16)
            nc.scalar.activation(a2[:], psf1[:], AF.Relu)
            zstat(a2[:], [64, 512], 6, "scrfc", "fc1")
            tstat(a2[:], [64, 512], 'tf', 6, None, "scrfc", "fc1")

            a2t = fcp.tile([128, 4, 64], F16)
            for k in range(4):
                ptr = pst.tile([128, 64], F16, name=f"ptr1_{k}", tag="psumT")
                nc.tensor.transpose(ptr[:], a2[:, k * 128:(k + 1) * 128],
                                    idsb[:])
                nc.scalar.copy(a2t[:, k, :], ptr[:])
            ag2in = dram.tile([4, 128, 64], F16)
            for k in range(4):
                nc.sync.dma_start(ag2in[k], a2t[:, k, :])
            ag2out = dram.tile([NCORES, 4, 128, 64], F16, addr_space="Shared")
            nc.gpsimd.collective_compute(
                "AllGather", ALU.bypass,
                replica_groups=[list(range(NCORES))],
                ins=[ag2in[:].opt()], outs=[ag2out[:].opt()])

            psf2 = psf.tile([64, 512], F32, name="psf2", tag="psumfc")
            for r in range(NCORES):
                for k in range(4):
                    lt = wfcp.tile([128, 64], F16, name=f"l2t_{r}_{k}",
                                   tag="l2t")
                    nc.sync.dma_start(lt[:], ag2out[r, k])
                    wt = wfcp.tile([128, 512], F16, name=f"w2t_{r}_{k}",
                                   tag="w2t")
                    nc.sync.dma_start(wt[:], w2fc[r * 4 + k])
                    nc.tensor.matmul(psf2[:], lt[:], wt[:],
                                     start=(r == 0 and k == 0), stop=False)
            nc.tensor.matmul(psf2[:], ones1[:], b2sb[:], start=False, stop=True)
            a3 = fcp.tile([64, 512], F16)
            nc.scalar.activation(a3[:], psf2[:], AF.Relu)
            zstat(a3[:], [64, 512], 7, "scrfc", "fc2")
            tstat(a3[:], [64, 512], 'tf', 7, None, "scrfc", "fc2")

            a3t = fcp.tile([128, 4, 64], F16)
            for k in range(4):
                ptr = pst.tile([128, 64], F16, name=f"ptr2_{k}", tag="psumT")
                nc.tensor.transpose(ptr[:], a3[:, k * 128:(k + 1) * 128],
                                    idsb[:])
                nc.scalar.copy(a3t[:, k, :], ptr[:])

            psf3 = pst.tile([64, 16], F32, name="psf3", tag="psum3")
            for k in range(4):
                nc.tensor.matmul(psf3[:, 0:10], a3t[:, k, :], w3fsb[:, k, :],
                                 start=(k == 0), stop=(k == 3))
            fc3sb = fcp.tile([64, 16], F32)
            nc.scalar.copy(fc3sb[:], psf3[:])
            nc.sync.dma_start(out[0:64, NSTAT:NSTAT + 16], fc3sb[:])
            nc.sync.dma_start(out[:, 0:NSTAT], stat[:])

    _g.pop('s1pool_tiles', None)
    nc.compile()
    assert len(man) <= NSTAT, len(man)
    return nc, man


def _fold_bn(p):
    W, Bv = {}, {}
    for l in (1, 2, 3, 4, 5):
        g = np.asarray(p[f'bn{l}_g'], np.float32)
        b = np.asarray(p[f'bn{l}_b'], np.float32)
        m = np.asarray(p[f'bn{l}_m'], np.float32)
        v = np.asarray(p[f'bn{l}_v'], np.float32)
        s = g / np.sqrt(v + BN_EPS)
        W[l] = np.asarray(p[f'conv{l}_w'], np.float32) * s[:, None, None, None]
        Bv[l] = (np.asarray(p[f'conv{l}_b'], np.float32) - m) * s + b
    return W, Bv


def _prep(x, params):
    x = np.asarray(x, np.float32)
    W, Bv = _fold_bn(params)

    w1p = W[1].transpose(2, 3, 1, 0).reshape(121, 3, 64).astype(np.float16)
    w2l = np.zeros((128, 13, 192), np.float16)
    for j, (sA, sB, kind) in enumerate(PAIRS2):
        w2l[0:64, j, :] = W[2][:, :, sA[0], sA[1]].T
        w2l[64:128, j, :] = W[2][:, :, sB[0], sB[1]].T
    w2l[0:64, 12, :] = W[2][:, :, 4, 4].T
    w3a = np.zeros((128, 9, 384), np.float16)
    w3b = np.zeros((128, 6, 384), np.float16)
    for kh in range(3):
        for kw in range(3):
            w3a[:, kh * 3 + kw, :] = W[3][:, 0:128, kh, kw].T
        w3b[0:64, kh * 2, :] = W[3][:, 128:192, kh, 0].T
        w3b[64:128, kh * 2, :] = W[3][:, 128:192, kh, 1].T
        w3b[0:64, kh * 2 + 1, :] = W[3][:, 128:192, kh, 2].T
    w4p = W[4].reshape(256, 3, 128, 3, 3).transpose(2, 1, 3, 4, 0) \
        .reshape(128, 27, 256).astype(np.float16)
    w5p = W[5].reshape(256, 2, 128, 3, 3).transpose(2, 1, 3, 4, 0) \
        .reshape(128, 18, 256).astype(np.float16)
    bconv = np.zeros((128, 10), np.float32)
    bconv[:, 0] = np.concatenate([Bv[1], Bv[1]])
    bconv[:, 1] = Bv[2][0:128]
    bconv[0:64, 2] = Bv[2][128:192]
    for m in range(3):
        bconv[:, 3 + m] = Bv[3][m * 128:(m + 1) * 128]
    for m in range(2):
        bconv[:, 6 + m] = Bv[4][m * 128:(m + 1) * 128]
        bconv[:, 8 + m] = Bv[5][m * 128:(m + 1) * 128]

    xp = np.pad(x, ((0, 0), (0, 0), (2, 2), (2, 2)))
    R = np.empty((64, 3, 121, 3025), np.float16)
    for kh in range(11):
        for kw in range(11):
            R[:, :, kh * 11 + kw, :] = \
                xp[:, :, kh:kh + 220:4, kw:kw + 220:4].reshape(64, 3, 3025)

    W1 = np.asarray(params['fc1_w'], np.float32)
    W2 = np.asarray(params['fc2_w'], np.float32)
    W3 = np.asarray(params['fc3_w'], np.float32)
    b1 = np.asarray(params['fc1_b'], np.float32)
    b2 = np.asarray(params['fc2_b'], np.float32)
    idm = np.eye(64, dtype=np.float16)

    in_maps = []
    for r in range(NCORES):
        W1r = W1[r * 512:(r + 1) * 512]
        w1fc = W1r.reshape(512, 2, 128, 36).transpose(3, 1, 2, 0) \
            .reshape(72, 128, 512).astype(np.float16)
        w2fc = W2[r * 512:(r + 1) * 512].T.reshape(32, 128, 512) \
            .astype(np.float16)
        w3fc = W3[:, r * 512:(r + 1) * 512].T.reshape(4, 128, 10) \
            .transpose(1, 0, 2).astype(np.float16).copy()
        in_maps.append({
            "xflat": x[r * 8:(r + 1) * 8].reshape(128, 9408).copy(),
            "r1": np.ascontiguousarray(R[r * 8:(r + 1) * 8]),
            "w1p": w1p, "w2l": w2l, "w3a": w3a, "w3b": w3b,
            "w4p": w4p, "w5p": w5p, "bconv": bconv,
            "w1fc": w1fc, "w2fc": w2fc, "w3fc": w3fc,
            "b1r": b1[r * 512:(r + 1) * 512].reshape(1, 512)
                .astype(np.float16),
            "b2r": b2[r * 512:(r + 1) * 512].reshape(1, 512)
                .astype(np.float16),
            "idm": idm,
        })
    return in_maps


def _get_runner():
    """Build (once) a cached jitted SPMD executor for the compiled program."""
    if 'runner' in _g:
        return _g['runner']
    import jax
    from jax.sharding import Mesh, PartitionSpec
    try:
        from jax.experimental.shard_map import shard_map
    except ImportError:
        from jax.shard_map import shard_map
    from concourse import bass2jax

    nc, man = _g['nc'], _g['man']
    bass2jax.install_neuronx_cc_hook()

    partition_name = (nc.partition_id_tensor.name
                      if nc.partition_id_tensor else None)
    in_names, out_names, out_avals, zero_outs = [], [], [], []
    for alloc in nc.m.functions[0].allocations:
        if not isinstance(alloc, mybir.MemoryLocationSet):
            continue
        name = alloc.memorylocations[0].name
        if alloc.kind == "ExternalInput":
            if name != partition_name:
                in_names.append(name)
        elif alloc.kind == "ExternalOutput":
            out_names.append(name)
            shape = tuple(alloc.tensor_shape)
            dtype = mybir.dt.np(alloc.dtype)
            out_avals.append(jax.core.ShapedArray(shape, dtype))
            zero_outs.append(np.zeros(shape, dtype))
    n_params = len(in_names)
    n_outs = len(out_avals)
    all_names = in_names + out_names
    if partition_name is not None:
        all_names.append(partition_name)
    donate = tuple(range(n_params, n_params + n_outs))

    def _body(*args):
        operands = list(args)
        if partition_name is not None:
            operands.append(bass2jax.partition_id_tensor())
        outs = bass2jax._bass_exec_p.bind(
            *operands, out_avals=tuple(out_avals), in_names=tuple(all_names),
            out_names=tuple(out_names), lowering_input_output_aliases=(),
            sim_require_finite=True, sim_require_nnan=True, nc=nc)
        return tuple(outs)

    devices = jax.devices()[:NCORES]
    mesh = Mesh(np.asarray(devices), ("core",))
    in_specs = (PartitionSpec("core"),) * (n_params + n_outs)
    out_specs = (PartitionSpec("core"),) * n_outs
    sharded = jax.jit(
        shard_map(_body, mesh=mesh, in_specs=in_specs, out_specs=out_specs,
                  check_rep=False),
        donate_argnums=donate, keep_unused=True)

    def run(in_maps):
        concat_in = [np.concatenate([np.asarray(in_maps[c][n])
                                     for c in range(NCORES)], axis=0)
                     for n in in_names]
        concat_zero = [np.zeros((NCORES * z.shape[0], *z.shape[1:]), z.dtype)
                       for z in zero_outs]
        arrs = sharded(*concat_in, *concat_zero)
        return [{n: np.asarray(arrs[i]).reshape(NCORES, *out_avals[i].shape)[c]
                 for i, n in enumerate(out_names)} for c in range(NCORES)]

    _g['runner'] = run
    return run


def _assemble(results, man, b3):
    zeros = np.zeros(8, np.float64)
    tanh = np.zeros((7, 64), np.float64)
    logits = np.zeros((64, 10), np.float64)
    for r in range(NCORES):
        o = np.asarray(results[r]["out"], np.float64)
        stats = o[:, :NSTAT]
        logits += o[0:64, NSTAT:NSTAT + 10]
        for c, (kind, stage, extra) in enumerate(man):
            cv = stats[:, c]
            if kind == 'z':
                zeros[stage] += cv.sum()
            elif kind == 'tp':
                ia, ib = extra
                tanh[stage - 1, r * 8 + ia] += cv[0:64].sum()
                tanh[stage - 1, r * 8 + ib] += cv[64:128].sum()
            elif kind == 'ts':
                tanh[stage - 1, r * 8 + extra] += cv.sum()
            elif kind == 'tf':
                tanh[5 if stage == 6 else 6, :] += cv[0:64]
    logits += np.asarray(b3, np.float64)[None, :]
    return (logits.astype(np.float32), tanh.astype(np.float32),
            np.rint(zeros).astype(np.int32), SIZES.copy())


def kernel(x, params):
    if 'nc' not in _g:
        _g['nc'], _g['man'] = _build()
    in_maps = _prep(x, params)
    run = _get_runner()
    results = run(in_maps)
    return _assemble(results, _g['man'], np.asarray(params['fc3_b'],
                                                    np.float32))


# revision 30
# speedup vs baseline: 3879.9888x; 3879.9888x over previous
import sys, os

sys.path.insert(0, "/opt/trn_rl_repo")
import numpy as np

import concourse.bass as bass
import concourse.mybir as mybir
import concourse.bacc as bacc
import concourse.tile as tile

NCORES = 8
B = 8  # images per core
F16, F32 = mybir.dt.float16, mybir.dt.float32
AF, ALU = mybir.ActivationFunctionType, mybir.AluOpType
BETA = 0.5
BN_EPS = 1e-5

TRIPLES = [(0, 3), (3, 3), (6, 2)]
# conv2 5x5 shift pairing: ((khA,kwA),(khB,kwB), which shifted-copy tensor)
# 'b': partitions 64..127 hold input shifted by +1 flat element
# 'c': partitions 64..127 hold input shifted by +27 flat elements (= +1 row -4 cols)
PAIRS2 = [
    ((0, 0), (0, 1), 'b'), ((0, 2), (0, 3), 'b'), ((0, 4), (1, 0), 'c'),
    ((1, 1), (1, 2), 'b'), ((1, 3), (1, 4), 'b'),
    ((2, 0), (2, 1), 'b'), ((2, 2), (2, 3), 'b'), ((2, 4), (3, 0), 'c'),
    ((3, 1), (3, 2), 'b'), ((3, 3), (3, 4), 'b'),
    ((4, 0), (4, 1), 'b'), ((4, 2), (4, 3), 'b'),
]  # + single (4,4) at tile index 12
CH1 = [(0, 512), (512, 512), (1024, 512), (1536, 512), (2048, 512), (2560, 465)]
ROWS2 = [(0, 14), (14, 13)]
NSTAT = 224
NOUT = NSTAT + 16

SIZES = np.array([64 * 3 * 224 * 224, 64 * 64 * 27 * 27, 64 * 192 * 13 * 13,
                  64 * 384 * 13 * 13, 64 * 256 * 13 * 13, 64 * 256 * 36,
                  64 * 4096, 64 * 4096], dtype=np.int32)

_g = {}


def _win(t_ap, pstart, pcount, off, dims):
    """Custom AP: partition range + arbitrary (step,count) free dims."""
    pstep = t_ap.ap[0][0]
    return bass.AP(tensor=t_ap.tensor, offset=t_ap.offset + pstart * pstep + off,
                   ap=[[pstep, pcount]] + [list(d) for d in dims])


def _build(sim1=False):
    nc = bacc.Bacc("TRN2", target_bir_lowering=False, debug=False,
                   num_devices=1 if sim1 else NCORES)
    man = []

    def col(kind, stage, extra=None):
        man.append((kind, stage, extra))
        return len(man) - 1

    def P(n, s, d):
        return nc.dram_tensor(n, s, d, kind="ExternalInput").ap()

    xflat = P("xflat", [128, 9408], F32)
    r1 = P("r1", [B, 3, 121, 3025], F16)
    w1p = P("w1p", [121, 3, 64], F16)
    w2l = P("w2l", [128, 13, 192], F16)
    w3a = P("w3a", [128, 9, 384], F16)
    w3b = P("w3b", [128, 6, 384], F16)
    w4p = P("w4p", [128, 27, 256], F16)
    w5p = P("w5p", [128, 18, 256], F16)
    bconv = P("bconv", [128, 10], F32)
    w1fc = P("w1fc", [72, 128, 512], F16)
    w2fc = P("w2fc", [32, 128, 512], F16)
    w3fc = P("w3fc", [128, 4, 10], F16)
    b1r = P("b1r", [1, 512], F16)
    b2r = P("b2r", [1, 512], F16)
    idm = P("idm", [64, 64], F16)
    out = nc.dram_tensor("out", [128, NOUT], F32, kind="ExternalOutput").ap()

    with tile.TileContext(nc) as tc:
        with (tc.tile_pool(name="wsb", bufs=1) as wsb,
              tc.tile_pool(name="rp", bufs=6) as rp,
              tc.tile_pool(name="s1", bufs=2) as s1p,
              tc.tile_pool(name="p2", bufs=2) as p2p,
              tc.tile_pool(name="s2", bufs=2) as s2p,
              tc.tile_pool(name="p3", bufs=2) as p3p,
              tc.tile_pool(name="p45", bufs=2) as p45p,
              tc.tile_pool(name="scr", bufs=2) as scrp,
              tc.tile_pool(name="fc", bufs=1) as fcp,
              tc.tile_pool(name="wfc", bufs=6) as wfcp,
              tc.tile_pool(name="ps", bufs=4, space="PSUM") as ps,
              tc.tile_pool(name="psf", bufs=1, space="PSUM") as psf,
              tc.tile_pool(name="pst", bufs=1, space="PSUM") as pst,
              tc.tile_pool(name="dram", bufs=1, space="DRAM") as dram):

            # resident weights / consts
            w1sb = wsb.tile([121, 3, 64], F16); nc.sync.dma_start(w1sb[:], w1p)
            w2sb = wsb.tile([128, 13, 192], F16)
            w3asb = wsb.tile([128, 9, 384], F16)
            w3bsb = wsb.tile([128, 6, 384], F16)
            w4sb = wsb.tile([128, 27, 256], F16)
            w5sb = wsb.tile([128, 18, 256], F16)
            bcsb = wsb.tile([128, 10], F32); nc.sync.dma_start(bcsb[:], bconv)
            idsb = wsb.tile([64, 64], F16); nc.sync.dma_start(idsb[:], idm)
            b1sb = wsb.tile([1, 512], F16); nc.sync.dma_start(b1sb[:], b1r)
            b2sb = wsb.tile([1, 512], F16); nc.sync.dma_start(b2sb[:], b2r)
            w3fsb = wsb.tile([128, 4, 10], F16); nc.sync.dma_start(w3fsb[:], w3fc)
            ones1 = wsb.tile([1, 64], F16); nc.vector.memset(ones1[:], 1.0)
            stat = wsb.tile([128, NSTAT], F32); nc.vector.memset(stat[:], 0.0)
            pool5sb = []
            for m in range(2):
                t = wsb.tile([128, B, 36], F16, name=f"pool5_{m}")
                pool5sb.append(t)

            def zstat(ap_in, shape, stage, tag, nm):
                s = scrp.tile(shape, F16, name=f"z_{nm}", tag=tag)
                c = col('z', stage)
                nc.vector.tensor_scalar(s[:], ap_in, 0.0, None, ALU.is_equal,
                                        op1=ALU.add,
                                        accum_out=stat[0:shape[0], c:c + 1])

            def tstat(ap_in, shape, kind, stage, extra, tag, nm):
                s = scrp.tile(shape, F16, name=f"t_{nm}", tag=tag)
                c = col(kind, stage, extra)
                nc.scalar.activation(s[:], ap_in, AF.Tanh, scale=BETA,
                                     accum_out=stat[0:shape[0], c:c + 1])

            # ---- phase emitters (explicit software pipelining) ----
            S = {}

            def xcount():
                for i in range(4):
                    xc = rp.tile([128, 2352], F32, name=f"xc{i}", tag="xc",
                                 bufs=2)
                    nc.sync.dma_start(xc[:], xflat[:, i * 2352:(i + 1) * 2352])
                    c = col('z', 0)
                    # in-place compare (overwrite the loaded chunk)
                    nc.vector.tensor_scalar(xc[:], xc[:], 0.0, None,
                                            ALU.is_equal, op1=ALU.add,
                                            accum_out=stat[:, c:c + 1])

            def ph_conv1(pair):
                a, b_ = 2 * pair, 2 * pair + 1
                rts = {}
                for h, (h0, hn) in enumerate(((0, 1536), (1536, 1489))):
                    for (im, plo) in ((a, 0), (b_, 64)):
                        for c in range(3):
                            rt = rp.tile([121, 1536], F16,
                                         name=f"r_{im}_{c}_{h}", tag="rt",
                                         bufs=12)
                            nc.sync.dma_start(rt[:, 0:hn],
                                              r1[im, c, :, h0:h0 + hn])
                            rts[(plo, c, h)] = rt
                s1pair = s1p.tile([128, 3025], F16,
                                  name=f"s1pair{pair}", tag="s1pair")
                for ci, (nofs, nch) in enumerate(CH1):
                    h = ci // 3
                    ho = nofs - h * 1536
                    pt = ps.tile([128, 512], F32,
                                 name=f"ps1_{pair}_{nofs}", tag="psum")
                    for (im, plo) in ((a, 0), (b_, 64)):
                        for c in range(3):
                            nc.tensor.matmul(
                                pt[plo:plo + 64, 0:nch],
                                w1sb[:, c, :],
                                rts[(plo, c, h)][:, ho:ho + nch],
                                start=(c == 0), stop=(c == 2))
                    nc.scalar.activation(s1pair[:, nofs:nofs + nch],
                                         pt[:, 0:nch], AF.Relu,
                                         bias=bcsb[:, 0:1])
                v = s1pair[:].rearrange("q (y x) -> q y x", y=55)
                t1 = scrp.tile([128, 55, 27], F16,
                               name=f"t1_{pair}", tag="ptmp1")
                nc.vector.tensor_max(t1[:], v[:, :, 0:53:2], v[:, :, 1:54:2])
                nc.vector.tensor_max(t1[:], t1[:], v[:, :, 2:55:2])
                s1pool = s1p.tile([128, 27, 27], F16,
                                  name=f"s1pool{pair}", tag="s1pool", bufs=3)
                nc.vector.tensor_max(s1pool[:], t1[:, 0:53:2, :],
                                     t1[:, 1:54:2, :])
                nc.vector.tensor_max(s1pool[:], s1pool[:], t1[:, 2:55:2, :])
                s1f = s1pool[:].rearrange("q y x -> q (y x)")
                zstat(s1f, [128, 729], 1, "scr729", f"s1_{pair}")
                tstat(s1f, [128, 729], 'tp', 1, (a, b_), "scr729",
                      f"s1_{pair}")
                S[('s1pool', pair)] = s1pool

            def ph_p2(img):
                pair, lo = img // 2, (img % 2) * 64
                s1pool = S[('s1pool', pair)]
                p2b = p2p.tile([128, 31, 31], F16, name=f"p2b{img}", tag="p2b")
                p2c = p2p.tile([128, 31, 31], F16, name=f"p2c{img}", tag="p2c")
                nc.vector.memset(p2b[:], 0.0)
                nc.vector.memset(p2c[:], 0.0)
                src = s1pool[lo:lo + 64, :, :]
                nc.sync.dma_start(p2b[0:64, 2:29, 2:29], src)
                nc.sync.dma_start(p2c[0:64, 2:29, 2:29], src)
                # shifted copies built directly from s1pool (no serial chain):
                # p2b[64+c, f] = p2b[c, f+1]; p2c[64+c, f] = p2c[c, f+27]
                nc.sync.dma_start(p2b[64:128, 2:29, 1:28], src)
                nc.sync.dma_start(p2c[64:128, 1:28, 6:31], src[:, 0:27, 0:25])
                nc.sync.dma_start(p2c[64:128, 2:29, 0:2], src[:, 0:27, 25:27])
                p2bf = p2b[:].rearrange("q y x -> q (y x)")
                p2cf = p2c[:].rearrange("q y x -> q (y x)")
                S[('p2', img)] = (p2bf, p2cf)

            def ph_conv2(img):
                p2bf, p2cf = S[('p2', img)]
                s2a = s2p.tile([128, 27, 27], F16, name=f"s2a{img}", tag="s2a")
                s2b = s2p.tile([64, 27, 27], F16, name=f"s2b{img}", tag="s2b")
                for (mlo, msz, mtile, bcol) in ((0, 128, s2a, 1),
                                                (128, 64, s2b, 2)):
                    for (r0, nr) in ROWS2:
                        pt = ps.tile([128, 378], F32,
                                     name=f"ps2_{img}_{mlo}_{r0}", tag="psum")
                        for j, (sA, sB, kind) in enumerate(PAIRS2):
                            khA, kwA = sA
                            off = khA * 31 + kwA + r0 * 31
                            sf = p2bf if kind == 'b' else p2cf
                            nc.tensor.matmul(
                                pt[0:msz, 0:nr * 27],
                                w2sb[:, j, mlo:mlo + msz],
                                _win(sf, 0, 128, off, [(31, nr), (1, 27)]),
                                start=(j == 0), stop=False)
                        off = 4 * 31 + 4 + r0 * 31
                        nc.tensor.matmul(
                            pt[0:msz, 0:nr * 27],
                            w2sb[0:64, 12, mlo:mlo + msz],
                            _win(p2bf, 0, 64, off, [(31, nr), (1, 27)]),
                            start=False, stop=True)
                        nc.scalar.activation(
                            mtile[0:msz, r0:r0 + nr, :],
                            pt[0:msz, 0:nr * 27].rearrange(
                                "q (a b) -> q a b", a=nr),
                            AF.Relu, bias=bcsb[0:msz, bcol:bcol + 1])
                S[('s2', img)] = (s2a, s2b)

            def ph_pool2(img):
                tri = 0 if img < 3 else (1 if img < 6 else 2)
                t0, nimg = TRIPLES[tri]
                iml = img - t0
                s2a, s2b = S[('s2', img)]
                if iml == 0:
                    p3a = p3p.tile([128, nimg, 15, 15], F16,
                                   name=f"p3a_{tri}", tag="p3a")
                    p3b = p3p.tile([128, nimg, 15, 15], F16,
                                   name=f"p3b_{tri}", tag="p3b")
                    nc.vector.memset(p3a[:], 0.0)
                    nc.vector.memset(p3b[:], 0.0)
                    S[('p3', tri)] = (p3a, p3b)
                p3a, p3b = S[('p3', tri)]
                for (mtile, msz, p3x) in ((s2a, 128, p3a), (s2b, 64, p3b)):
                    t2 = scrp.tile([128, 27, 13], F16,
                                   name=f"t2_{img}_{msz}", tag="ptmp2")
                    nc.vector.tensor_max(t2[0:msz], mtile[:, :, 0:25:2],
                                         mtile[:, :, 1:26:2])
                    nc.vector.tensor_max(t2[0:msz], t2[0:msz],
                                         mtile[:, :, 2:27:2])
                    dst = p3x[0:msz, iml, 1:14, 1:14]
                    nc.vector.tensor_max(dst, t2[0:msz, 0:25:2, :],
                                         t2[0:msz, 1:26:2, :])
                    nc.vector.tensor_max(dst, dst, t2[0:msz, 2:27:2, :])
                    iv = p3x[0:msz, iml, 1:14, 1:14]
                    zstat(iv, [msz, 13, 13], 2, "scr169", f"s2_{img}_{msz}")
                    tstat(iv, [msz, 13, 13], 'ts', 2, img, "scr169",
                          f"s2_{img}_{msz}")

            def ph_conv345(tri):
                t0, nimg = TRIPLES[tri]
                p3a, p3b = S[('p3', tri)]
                p3bf = p3b[:].rearrange("q a y x -> q (a y x)")
                nc.sync.dma_start(p3bf[64:128, 0:nimg * 225 - 1],
                                  p3bf[0:64, 1:nimg * 225])
                p3af = p3a[:].rearrange("q a y x -> q (a y x)")
                AP3 = [(225, nimg), (15, 13), (1, 13)]
                N3 = nimg * 169

                p4s = []
                for m in range(3):
                    p4x = p45p.tile([128, nimg, 15, 15], F16,
                                    name=f"p4_{tri}_{m}", tag=f"p4_{m}")
                    nc.vector.memset(p4x[:], 0.0)
                    p4s.append(p4x)
                for m in range(3):
                    pt = ps.tile([128, 512], F32,
                                 name=f"ps3_{tri}_{m}", tag="psum")
                    ms = slice(m * 128, (m + 1) * 128)
                    for kh in range(3):
                        for kw in range(3):
                            nc.tensor.matmul(
                                pt[:, 0:N3], w3asb[:, kh * 3 + kw, ms],
                                _win(p3af, 0, 128, kh * 15 + kw, AP3),
                                start=(kh == 0 and kw == 0), stop=False)
                    for kh in range(3):
                        nc.tensor.matmul(
                            pt[:, 0:N3], w3bsb[:, kh * 2, ms],
                            _win(p3bf, 0, 128, kh * 15 + 0, AP3),
                            start=False, stop=False)
                        nc.tensor.matmul(
                            pt[:, 0:N3], w3bsb[0:64, kh * 2 + 1, ms],
                            _win(p3bf, 0, 64, kh * 15 + 2, AP3),
                            start=False, stop=(kh == 2))
                    nc.scalar.activation(
                        p4s[m][:, :, 1:14, 1:14],
                        pt[:, 0:N3].rearrange("q (a y x) -> q a y x",
                                              a=nimg, y=13),
                        AF.Relu, bias=bcsb[:, 3 + m:4 + m])
                    for i2 in range(nimg):
                        iv = p4s[m][:, i2, 1:14, 1:14]
                        zstat(iv, [128, 13, 13], 3, "scr169",
                              f"s3_{tri}_{m}_{i2}")
                        tstat(iv, [128, 13, 13], 'ts', 3, t0 + i2,
                              "scr169", f"s3_{tri}_{m}_{i2}")

                p5s = []
                for m in range(2):
                    p5x = p45p.tile([128, nimg, 15, 15], F16,
                                    name=f"p5_{tri}_{m}", tag=f"p5_{m}")
                    nc.vector.memset(p5x[:], 0.0)
                    p5s.append(p5x)
                p4fs = [p[:].rearrange("q a y x -> q (a y x)") for p in p4s]
                for m in range(2):
                    pt = ps.tile([128, 512], F32,
                                 name=f"ps4_{tri}_{m}", tag="psum")
                    ms = slice(m * 128, (m + 1) * 128)
                    n = 0
                    for cb in range(3):
                        for kh in range(3):
                            for kw in range(3):
                                nc.tensor.matmul(
                                    pt[:, 0:N3],
                                    w4sb[:, cb * 9 + kh * 3 + kw, ms],
                                    _win(p4fs[cb], 0, 128, kh * 15 + kw, AP3),
                                    start=(n == 0), stop=(n == 26))
                                n += 1
                    nc.scalar.activation(
                        p5s[m][:, :, 1:14, 1:14],
                        pt[:, 0:N3].rearrange("q (a y x) -> q a y x",
                                              a=nimg, y=13),
                        AF.Relu, bias=bcsb[:, 6 + m:7 + m])
                    for i2 in range(nimg):
                        iv = p5s[m][:, i2, 1:14, 1:14]
                        zstat(iv, [128, 13, 13], 4, "scr169",
                              f"s4_{tri}_{m}_{i2}")
                        tstat(iv, [128, 13, 13], 'ts', 4, t0 + i2,
                              "scr169", f"s4_{tri}_{m}_{i2}")

                p5fs = [p[:].rearrange("q a y x -> q (a y x)") for p in p5s]
                for m in range(2):
                    pt = ps.tile([128, 512], F32,
                                 name=f"ps5_{tri}_{m}", tag="psum")
                    ms = slice(m * 128, (m + 1) * 128)
                    n = 0
                    for cb in range(2):
                        for kh in range(3):
                            for kw in range(3):
                                nc.tensor.matmul(
                                    pt[:, 0:N3],
                                    w5sb[:, cb * 9 + kh * 3 + kw, ms],
                                    _win(p5fs[cb], 0, 128, kh * 15 + kw, AP3),
                                    start=(n == 0), stop=(n == 17))
                                n += 1
                    s5 = s2p.tile([128, nimg, 13, 13], F16,
                                  name=f"s5_{tri}_{m}", tag="s5")
                    nc.scalar.activation(
                        s5[:], pt[:, 0:N3].rearrange(
                            "q (a y x) -> q a y x", a=nimg, y=13),
                        AF.Relu, bias=bcsb[:, 8 + m:9 + m])
                    for i2 in range(nimg):
                        vv = s5[:, i2]
                        t5 = scrp.tile([128, 13, 6], F16,
                                       name=f"t5_{tri}_{m}_{i2}", tag="ptmp5")
                        nc.vector.tensor_max(t5[:], vv[:, :, 0:11:2],
                                             vv[:, :, 1:12:2])
                        nc.vector.tensor_max(t5[:], t5[:], vv[:, :, 2:13:2])
                        dst = pool5sb[m][:, t0 + i2, :].rearrange(
                            "q (y x) -> q y x", y=6)
                        nc.vector.tensor_max(dst, t5[:, 0:11:2, :],
                                             t5[:, 1:12:2, :])
                        nc.vector.tensor_max(dst, dst, t5[:, 2:13:2, :])
                        pv = pool5sb[m][:, t0 + i2, :]
                        zstat(pv, [128, 36], 5, "scr36", f"s5_{tri}_{m}_{i2}")
                        tstat(pv, [128, 36], 'ts', 5, t0 + i2, "scr36",
                              f"s5_{tri}_{m}_{i2}")

            # ---- pipelined phase order ----
            nc.sync.dma_start(w2sb[:], w2l)
            ph_conv1(0)
            ph_conv1(1)
            ph_p2(0)
            nc.sync.dma_start(w3asb[:], w3a)
            nc.sync.dma_start(w3bsb[:], w3b)
            ph_conv2(0); ph_pool2(0)
            ph_p2(1)
            nc.sync.dma_start(w4sb[:], w4p)
            ph_conv2(1); ph_pool2(1)
            ph_conv1(2)
            ph_p2(2)
            nc.sync.dma_start(w5sb[:], w5p)
            ph_conv2(2); ph_pool2(2)
            ph_p2(3); ph_conv2(3); ph_pool2(3)
            ph_conv1(3)
            ph_conv345(0)
            xcount()
            ph_p2(4); ph_conv2(4); ph_pool2(4)
            ph_p2(5); ph_conv2(5); ph_pool2(5)
            ph_p2(6); ph_conv2(6); ph_pool2(6)
            ph_conv345(1)
            ph_p2(7); ph_conv2(7); ph_pool2(7)
            ph_conv345(2)

            # ============ FC ============
            ag1in = dram.tile([2, 128, B, 36], F16)
            for m in range(2):
                for (t0, nimg) in TRIPLES:
                    nc.sync.dma_start(ag1in[m][:, t0:t0 + nimg, :],
                                      pool5sb[m][:, t0:t0 + nimg, :])
            ag1out = dram.tile([NCORES, 2, 128, B, 36], F16,
                               addr_space="Shared")
            if sim1:
                nc.sync.dma_start(ag1out[0], ag1in[:])
            else:
                nc.gpsimd.collective_compute(
                    "AllGather", ALU.bypass,
                    replica_groups=[list(range(NCORES))],
                    ins=[ag1in[:].opt()], outs=[ag1out[:].opt()])

            # xfc layout: [c_part, (cb, rank, img, s)] so the post-gather
            # copy is plain contiguous; fc1 lhsT chunks are strided views.
            xfc = fcp.tile([128, 2, NCORES, B, 36], F16)
            ag1b = ag1out[:]
            xb = xfc[:]
            for cb in range(2):
                srcap = bass.AP(
                    tensor=ag1b.tensor,
                    offset=ag1b.offset + cb * 128 * B * 36,
                    ap=[[B * 36, 128], [2 * 128 * B * 36, NCORES],
                        [1, B * 36]])
                dstap = bass.AP(
                    tensor=xb.tensor,
                    offset=xb.offset + cb * NCORES * B * 36,
                    ap=[[xb.ap[0][0], 128], [B * 36, NCORES], [1, B * 36]])
                nc.sync.dma_start(out=dstap, in_=srcap)

            psf1 = psf.tile([64, 512], F32, name="psf1", tag="psumfc")
            w1b = w1fc
            for g in range(18):
                wt = wfcp.tile([128, 4, 512], F16, name=f"w1t{g}", tag="w1t",
                               bufs=3)
                srcap = bass.AP(tensor=w1b.tensor,
                                offset=w1b.offset + g * 4 * 128 * 512,
                                ap=[[512, 128], [128 * 512, 4], [1, 512]])
                nc.sync.dma_start(wt[:], srcap)
                for j in range(4):
                    kc = 4 * g + j
                    s, cb = kc // 2, kc % 2
                    lhs = _win(xb, 0, 128, cb * NCORES * B * 36 + s,
                               [(B * 36, NCORES), (36, B)])
                    nc.tensor.matmul(psf1[:], lhs, wt[:, j, :],
                                     start=(kc == 0), stop=False)
            nc.tensor.matmul(psf1[:], ones1[:], b1sb[:], start=False, stop=True)
            a2 = fcp.tile([64, 512], F16)
            nc.scalar.activation(a2[:], psf1[:], AF.Relu)
            zstat(a2[:], [64, 512], 6, "scrfc", "fc1")
            tstat(a2[:], [64, 512], 'tf', 6, None, "scrfc", "fc1")

            a2t = fcp.tile([128, 4, 64], F16)
            for k in range(4):
                ptr = pst.tile([128, 64], F16, name=f"ptr1_{k}", tag="psumT")
                nc.tensor.transpose(ptr[:], a2[:, k * 128:(k + 1) * 128],
                                    idsb[:])
                nc.scalar.copy(a2t[:, k, :], ptr[:])
            ag2in = dram.tile([4, 128, 64], F16)
            for k in range(4):
                nc.sync.dma_start(ag2in[k], a2t[:, k, :])
            ag2out = dram.tile([NCORES, 4, 128, 64], F16, addr_space="Shared")
            if sim1:
                nc.sync.dma_start(ag2out[0], ag2in[:])
            else:
                nc.gpsimd.collective_compute(
                    "AllGather", ALU.bypass,
                    replica_groups=[list(range(NCORES))],
                    ins=[ag2in[:].opt()], outs=[ag2out[:].opt()])

            psf2 = psf.tile([64, 512], F32, name="psf2", tag="psumfc")
            ag2b = ag2out[:]
            for r in range(NCORES):
                lt = wfcp.tile([128, 4, 64], F16, name=f"l2t_{r}",
                               tag="l2t", bufs=4)
                lsrc = bass.AP(tensor=ag2b.tensor,
                               offset=ag2b.offset + r * 4 * 128 * 64,
                               ap=[[64, 128], [128 * 64, 4], [1, 64]])
                nc.sync.dma_start(lt[:], lsrc)
                wt = wfcp.tile([128, 4, 512], F16, name=f"w2t_{r}",
                               tag="w2t", bufs=3)
                wsrc = bass.AP(tensor=w2fc.tensor,
                               offset=w2fc.offset + r * 4 * 128 * 512,
                               ap=[[512, 128], [128 * 512, 4], [1, 512]])
                nc.sync.dma_start(wt[:], wsrc)
                for k in range(4):
                    nc.tensor.matmul(psf2[:], lt[:, k, :], wt[:, k, :],
                                     start=(r == 0 and k == 0), stop=False)
            nc.tensor.matmul(psf2[:], ones1[:], b2sb[:], start=False, stop=True)
            a3 = fcp.tile([64, 512], F16)
            nc.scalar.activation(a3[:], psf2[:], AF.Relu)
            zstat(a3[:], [64, 512], 7, "scrfc", "fc2")
            tstat(a3[:], [64, 512], 'tf', 7, None, "scrfc", "fc2")

            a3t = fcp.tile([128, 4, 64], F16)
            for k in range(4):
                ptr = pst.tile([128, 64], F16, name=f"ptr2_{k}", tag="psumT")
                nc.tensor.transpose(ptr[:], a3[:, k * 128:(k + 1) * 128],
                                    idsb[:])
                nc.scalar.copy(a3t[:, k, :], ptr[:])

            psf3 = pst.tile([64, 16], F32, name="psf3", tag="psum3")
            for k in range(4):
                nc.tensor.matmul(psf3[:, 0:10], a3t[:, k, :], w3fsb[:, k, :],
                                 start=(k == 0), stop=(k == 3))
            fc3sb = fcp.tile([64, 16], F32)
            nc.scalar.copy(fc3sb[:], psf3[:])
            nc.sync.dma_start(out[0:64, NSTAT:NSTAT + 16], fc3sb[:])
            nc.sync.dma_start(out[:, 0:NSTAT], stat[:])

    nc.compile()
    assert len(man) <= NSTAT, len(man)
    return nc, man


def _fold_bn(p):
    W, Bv = {}, {}
    for l in (1, 2, 3, 4, 5):
        g = np.asarray(p[f'bn{l}_g'], np.float32)
        b = np.asarray(p[f'bn{l}_b'], np.float32)
        m = np.asarray(p[f'bn{l}_m'], np.float32)
        v = np.asarray(p[f'bn{l}_v'], np.float32)
        s = g / np.sqrt(v + BN_EPS)
        W[l] = np.asarray(p[f'conv{l}_w'], np.float32) * s[:, None, None, None]
        Bv[l] = (np.asarray(p[f'conv{l}_b'], np.float32) - m) * s + b
    return W, Bv


def _prep(x, params):
    x = np.asarray(x, np.float32)
    W, Bv = _fold_bn(params)

    w1p = W[1].transpose(2, 3, 1, 0).reshape(121, 3, 64).astype(np.float16)
    w2l = np.zeros((128, 13, 192), np.float16)
    for j, (sA, sB, kind) in enumerate(PAIRS2):
        w2l[0:64, j, :] = W[2][:, :, sA[0], sA[1]].T
        w2l[64:128, j, :] = W[2][:, :, sB[0], sB[1]].T
    w2l[0:64, 12, :] = W[2][:, :, 4, 4].T
    w3a = np.zeros((128, 9, 384), np.float16)
    w3b = np.zeros((128, 6, 384), np.float16)
    for kh in range(3):
        for kw in range(3):
            w3a[:, kh * 3 + kw, :] = W[3][:, 0:128, kh, kw].T
        w3b[0:64, kh * 2, :] = W[3][:, 128:192, kh, 0].T
        w3b[64:128, kh * 2, :] = W[3][:, 128:192, kh, 1].T
        w3b[0:64, kh * 2 + 1, :] = W[3][:, 128:192, kh, 2].T
    w4p = W[4].reshape(256, 3, 128, 3, 3).transpose(2, 1, 3, 4, 0) \
        .reshape(128, 27, 256).astype(np.float16)
    w5p = W[5].reshape(256, 2, 128, 3, 3).transpose(2, 1, 3, 4, 0) \
        .reshape(128, 18, 256).astype(np.float16)
    bconv = np.zeros((128, 10), np.float32)
    bconv[:, 0] = np.concatenate([Bv[1], Bv[1]])
    bconv[:, 1] = Bv[2][0:128]
    bconv[0:64, 2] = Bv[2][128:192]
    for m in range(3):
        bconv[:, 3 + m] = Bv[3][m * 128:(m + 1) * 128]
    for m in range(2):
        bconv[:, 6 + m] = Bv[4][m * 128:(m + 1) * 128]
        bconv[:, 8 + m] = Bv[5][m * 128:(m + 1) * 128]

    xp = np.pad(x, ((0, 0), (0, 0), (2, 2), (2, 2)))
    R = np.empty((64, 3, 121, 3025), np.float16)
    for kh in range(11):
        for kw in range(11):
            R[:, :, kh * 11 + kw, :] = \
                xp[:, :, kh:kh + 220:4, kw:kw + 220:4].reshape(64, 3, 3025)

    W1 = np.asarray(params['fc1_w'], np.float32)
    W2 = np.asarray(params['fc2_w'], np.float32)
    W3 = np.asarray(params['fc3_w'], np.float32)
    b1 = np.asarray(params['fc1_b'], np.float32)
    b2 = np.asarray(params['fc2_b'], np.float32)
    idm = np.eye(64, dtype=np.float16)

    in_maps = []
    for r in range(NCORES):
        W1r = W1[r * 512:(r + 1) * 512]
        w1fc = W1r.reshape(512, 2, 128, 36).transpose(3, 1, 2, 0) \
            .reshape(72, 128, 512).astype(np.float16)
        w2fc = W2[r * 512:(r + 1) * 512].T.reshape(32, 128, 512) \
            .astype(np.float16)
        w3fc = W3[:, r * 512:(r + 1) * 512].T.reshape(4, 128, 10) \
            .transpose(1, 0, 2).astype(np.float16).copy()
        in_maps.append({
            "xflat": x[r * 8:(r + 1) * 8].reshape(128, 9408).copy(),
            "r1": np.ascontiguousarray(R[r * 8:(r + 1) * 8]),
            "w1p": w1p, "w2l": w2l, "w3a": w3a, "w3b": w3b,
            "w4p": w4p, "w5p": w5p, "bconv": bconv,
            "w1fc": w1fc, "w2fc": w2fc, "w3fc": w3fc,
            "b1r": b1[r * 512:(r + 1) * 512].reshape(1, 512)
                .astype(np.float16),
            "b2r": b2[r * 512:(r + 1) * 512].reshape(1, 512)
                .astype(np.float16),
            "idm": idm,
        })
    return in_maps


def _get_runner():
    """Build (once) a cached jitted SPMD executor for the compiled program."""
    if 'runner' in _g:
        return _g['runner']
    rp = _make_jit(_g['nc'])
    in_names, out_names = rp['in_names'], rp['out_names']
    out_avals, zero_outs = rp['out_avals'], rp['zero_outs']
    sharded = rp['sharded']

    def run(in_maps):
        concat_in = [np.concatenate([np.asarray(in_maps[c][n])
                                     for c in range(NCORES)], axis=0)
                     for n in in_names]
        concat_zero = [np.zeros((NCORES * z.shape[0], *z.shape[1:]), z.dtype)
                       for z in zero_outs]
        arrs = sharded(*concat_in, *concat_zero)
        return [{n: np.asarray(arrs[i]).reshape(NCORES, *out_avals[i].shape)[c]
                 for i, n in enumerate(out_names)} for c in range(NCORES)]

    _g['rparts'] = rp
    _g['runner'] = run
    return run


def _make_jit(nc):
    import jax
    from jax.sharding import Mesh, PartitionSpec
    try:
        from jax.experimental.shard_map import shard_map
    except ImportError:
        from jax.shard_map import shard_map
    from concourse import bass2jax
    bass2jax.install_neuronx_cc_hook()

    partition_name = (nc.partition_id_tensor.name
                      if nc.partition_id_tensor else None)
    in_names, out_names, out_avals, zero_outs = [], [], [], []
    for alloc in nc.m.functions[0].allocations:
        if not isinstance(alloc, mybir.MemoryLocationSet):
            continue
        name = alloc.memorylocations[0].name
        if alloc.kind == "ExternalInput":
            if name != partition_name:
                in_names.append(name)
        elif alloc.kind == "ExternalOutput":
            out_names.append(name)
            shape = tuple(alloc.tensor_shape)
            dtype = mybir.dt.np(alloc.dtype)
            out_avals.append(jax.core.ShapedArray(shape, dtype))
            zero_outs.append(np.zeros(shape, dtype))
    n_params, n_outs = len(in_names), len(out_avals)
    all_names = list(in_names) + out_names
    if partition_name is not None:
        all_names.append(partition_name)

    def _body(*args):
        operands = list(args)
        if partition_name is not None:
            operands.append(bass2jax.partition_id_tensor())
        outs = bass2jax._bass_exec_p.bind(
            *operands, out_avals=tuple(out_avals), in_names=tuple(all_names),
            out_names=tuple(out_names), lowering_input_output_aliases=(),
            sim_require_finite=True, sim_require_nnan=True, nc=nc)
        return tuple(outs)

    devices = jax.devices()[:NCORES]
    mesh = Mesh(np.asarray(devices), ("core",))
    sharded = jax.jit(
        shard_map(_body, mesh=mesh,
                  in_specs=(PartitionSpec("core"),) * (n_params + n_outs),
                  out_specs=(PartitionSpec("core"),) * n_outs,
                  check_rep=False),
        donate_argnums=tuple(range(n_params, n_params + n_outs)),
        keep_unused=True)
    return dict(sharded=sharded, in_names=in_names, out_names=out_names,
                out_avals=out_avals, zero_outs=zero_outs, n_params=n_params,
                n_outs=n_outs, mesh=mesh, body=_body,
                partition_name=partition_name)


def _null_nc():
    """Minimal program with the same external I/O shape as the real one."""
    nc = bacc.Bacc("TRN2", target_bir_lowering=False, debug=False,
                   num_devices=NCORES)
    dummy = nc.dram_tensor("nullin", [128, 8], F32, kind="ExternalInput").ap()
    out = nc.dram_tensor("out", [128, NOUT], F32, kind="ExternalOutput").ap()
    with tile.TileContext(nc) as tc:
        with tc.tile_pool(name="sb", bufs=1) as sb:
            t = sb.tile([128, 8], F32)
            nc.sync.dma_start(t[:], dummy)
            nc.sync.dma_start(out[:, 0:8], t[:])
    nc.compile()
    return nc


def _bench_jit(rp, dev_in, iters):
    import jax, time as _time
    from jax.sharding import NamedSharding, PartitionSpec
    sh = NamedSharding(rp['mesh'], PartitionSpec("core"))
    ts = []
    for _ in range(iters + 1):
        zs = [jax.device_put(
            np.zeros((NCORES * z.shape[0], *z.shape[1:]), z.dtype), sh)
            for z in rp['zero_outs']]
        jax.block_until_ready(zs)
        t0 = _time.time()
        out = rp['sharded'](*dev_in, *zs)
        jax.block_until_ready(out)
        ts.append(_time.time() - t0)
    return ts[1:]


def _timed(in_maps, iters=6):
    """Estimate on-device exec time: wall(full) - wall(null) with
    device-resident inputs."""
    import jax
    from jax.sharding import NamedSharding, PartitionSpec

    _get_runner()
    rp = _g['rparts']
    sh = NamedSharding(rp['mesh'], PartitionSpec("core"))
    dev_in = [jax.device_put(
        np.concatenate([np.asarray(in_maps[c][n]) for c in range(NCORES)],
                       axis=0), sh) for n in rp['in_names']]
    if 'null_rp' not in _g:
        _g['null_rp'] = _make_jit(_null_nc())
    nrp = _g['null_rp']
    nsh_in = [jax.device_put(
        np.zeros((NCORES * 128, 8), np.float32), sh)]

    tn = _bench_jit(nrp, nsh_in, iters)
    tf = _bench_jit(rp, dev_in, iters)
    t0, t1 = min(tn), min(tf)
    return dict(t_null=t0, t_full=t1, all_null=tn, all_full=tf,
                exec_s=t1 - t0)


def _assemble(results, man, b3):
    zeros = np.zeros(8, np.float64)
    tanh = np.zeros((7, 64), np.float64)
    logits = np.zeros((64, 10), np.float64)
    for r in range(NCORES):
        o = np.asarray(results[r]["out"], np.float64)
        stats = o[:, :NSTAT]
        logits += o[0:64, NSTAT:NSTAT + 10]
        for c, (kind, stage, extra) in enumerate(man):
            cv = stats[:, c]
            if kind == 'z':
                zeros[stage] += cv.sum()
            elif kind == 'tp':
                ia, ib = extra
                tanh[stage - 1, r * 8 + ia] += cv[0:64].sum()
                tanh[stage - 1, r * 8 + ib] += cv[64:128].sum()
            elif kind == 'ts':
                tanh[stage - 1, r * 8 + extra] += cv.sum()
            elif kind == 'tf':
                tanh[5 if stage == 6 else 6, :] += cv[0:64]
    logits += np.asarray(b3, np.float64)[None, :]
    return (logits.astype(np.float32), tanh.astype(np.float32),
            np.rint(zeros).astype(np.int32), SIZES.copy())


def kernel(x, params):
    if 'nc' not in _g:
        _g['nc'], _g['man'] = _build()
    in_maps = _prep(x, params)
    run = _get_runner()
    results = run(in_maps)
    return _assemble(results, _g['man'], np.asarray(params['fc3_b'],
                                                    np.float32))
